# revision 3
# baseline (speedup 1.0000x reference)
"""Multi-head graph attention (GATConv) Trainium2 Bass kernel, v3.

v2 -> v3: one-hot Mt/MT matrices are host-built and shipped as fp8 (PE
accepts fp8 lhsT with bf16 rhs), removing on-device one-hot construction;
per-edge [alpha_src | alpha_dst] land adjacently in one PSUM tile so one
ACT copy extracts both; LayerNorm's 1/sqrt uses the bit-trick rsqrt with
two Newton iterations on Pool/DVE so the ACT engine only ever runs Exp
(single activation-table load); per-window tile counts are variable.
"""
import sys
sys.path.insert(0, "/opt/trn_rl_repo")
import numpy as np
import ml_dtypes

import concourse.bacc as bacc
import concourse.mybir as mybir
import concourse.tile as tile
from concourse import bass_utils

BF = mybir.dt.bfloat16
F8 = mybir.dt.float8e4
F32 = mybir.dt.float32
I32 = mybir.dt.int32
AF = mybir.ActivationFunctionType
OP = mybir.AluOpType

MAGIC = 0x5F3759DF


class Cfg:
    def __init__(self, N, E, ncore=8):
        self.N = N
        self.E = E
        self.D = 128
        self.H = 4
        self.HD = 32
        self.ncore = ncore
        per = (N + ncore - 1) // ncore
        self.dshard = ((per + 127) // 128) * 128
        self.nwin = self.dshard // 128


FULL = Cfg(100_000, 1_600_000)


def build_program(cfg: Cfg, ntg):
    c = cfg
    ntg = list(ntg)
    assert len(ntg) == c.nwin
    base = np.zeros(c.nwin + 1, np.int64)
    np.cumsum(ntg, out=base[1:])
    S = int(base[-1]) * 128
    NTmax = max(ntg)

    nc = bacc.Bacc("TRN2", num_devices=c.ncore, debug=False)

    xeT = nc.dram_tensor("xeT", [128, S], BF, kind="ExternalInput")
    mt8 = nc.dram_tensor("mt8", [128, S], F8, kind="ExternalInput")
    mtg8 = nc.dram_tensor("mtg8", [128, S], F8, kind="ExternalInput")
    x_f = nc.dram_tensor("x_f", [c.dshard, 128], F32, kind="ExternalInput")
    xsT = nc.dram_tensor("xsT", [128, c.dshard], BF, kind="ExternalInput")
    wcs = nc.dram_tensor("wcs", [128, 132], BF, kind="ExternalInput")
    wdb = nc.dram_tensor("wdb", [128, 4], BF, kind="ExternalInput")
    prj = nc.dram_tensor("prj", [128, 128], BF, kind="ExternalInput")
    pb1 = nc.dram_tensor("pb1", [1, 128], BF, kind="ExternalInput")
    one1 = nc.dram_tensor("one1", [1, 128], BF, kind="ExternalInput")
    i128 = nc.dram_tensor("i128", [128, 128], BF, kind="ExternalInput")
    gb = nc.dram_tensor("gb", [128, 128], F32, kind="ExternalInput")
    bb = nc.dram_tensor("bb", [128, 128], F32, kind="ExternalInput")
    out = nc.dram_tensor("out", [c.dshard, 128], F32, kind="ExternalOutput")

    with tile.TileContext(nc) as tc:
        with tc.tile_pool(name="const", bufs=1) as cp:
            wcs_sb = cp.tile([128, 132], BF)
            nc.sync.dma_start(wcs_sb[:], wcs[:])
            wdb_sb = cp.tile([128, 4], BF)
            nc.sync.dma_start(wdb_sb[:], wdb[:])
            prj_sb = cp.tile([128, 128], BF)
            nc.sync.dma_start(prj_sb[:], prj[:])
            pb1_sb = cp.tile([1, 128], BF)
            nc.sync.dma_start(pb1_sb[:], pb1[:])
            one1_sb = cp.tile([1, 128], BF)
            nc.sync.dma_start(one1_sb[:], one1[:])
            i128_sb = cp.tile([128, 128], BF)
            nc.sync.dma_start(i128_sb[:], i128[:])
            gb_sb = cp.tile([128, 128], F32)
            nc.sync.dma_start(gb_sb[:], gb[:])
            bb_sb = cp.tile([128, 128], F32)
            nc.sync.dma_start(bb_sb[:], bb[:])
            xsT_sb = cp.tile([128, c.dshard], BF)
            nc.sync.dma_start(xsT_sb[:], xsT[:])

            with (
                tc.tile_pool(name="win", bufs=2) as wp,
                tc.tile_pool(name="sc", bufs=2) as sp,
                tc.tile_pool(name="psX", bufs=3, space="PSUM") as psX_p,
                tc.tile_pool(name="psA", bufs=1, space="PSUM") as psA_p,
                tc.tile_pool(name="psW", bufs=2, space="PSUM") as psW_p,
                tc.tile_pool(name="psS", bufs=1, space="PSUM") as psS_p,
                tc.tile_pool(name="psO", bufs=1, space="PSUM") as psO_p,
            ):
                for g in range(c.nwin):
                    NT = ntg[g]
                    sl = slice(int(base[g]) * 128, int(base[g + 1]) * 128)
                    xe = wp.tile([128, NTmax * 128], BF, tag="xe")
                    nc.sync.dma_start(xe[:, 0:NT * 128], xeT[:, sl])
                    mts = wp.tile([128, NTmax * 128], F8, tag="mts")
                    nc.sync.dma_start(mts[:, 0:NT * 128], mt8[:, sl])
                    mtgs = wp.tile([128, NTmax * 128], F8, tag="mtgs")
                    nc.sync.dma_start(mtgs[:, 0:NT * 128], mtg8[:, sl])
                    xw = wp.tile([128, 128], F32, tag="xw")
                    nc.sync.dma_start(xw[:], x_f[g * 128:(g + 1) * 128, :])

                    # alpha_dst per owned dst node: [c, 4]
                    psA = psA_p.tile([128, 4], F32, space="PSUM", tag="psA")
                    nc.tensor.matmul(psA[:], lhsT=xsT_sb[:, g * 128:(g + 1) * 128],
                                     rhs=wdb_sb[:], start=True, stop=True)
                    adb = sp.tile([128, 4], BF, tag="adb")
                    nc.vector.tensor_copy(adb[:], psA[:])

                    # 3 tiles share one PSUM bank ([xp|as|ad] x3) so a single
                    # strided copy extracts xp (DVE) and [as|ad] (ACT)
                    xp_sb = wp.tile([128, NTmax * 128], BF, tag="xp_sb")
                    asad = wp.tile([128, NTmax * 8], F32, tag="asad")
                    for t0 in range(0, NT, 3):
                        m = min(3, NT - t0)
                        ps_x = psX_p.tile([128, 408], F32, space="PSUM", tag="ps_x")
                        p3 = ps_x[:].rearrange("p (j f) -> p j f", f=136)
                        for j in range(m):
                            t = t0 + j
                            nc.tensor.matmul(ps_x[:, j * 136:j * 136 + 132],
                                             lhsT=xe[:, t * 128:(t + 1) * 128],
                                             rhs=wcs_sb[:], start=True, stop=True)
                            nc.tensor.matmul(ps_x[:, j * 136 + 132:j * 136 + 136],
                                             lhsT=mtgs[:, t * 128:(t + 1) * 128],
                                             rhs=adb[:], start=True, stop=True)
                        nc.vector.tensor_copy(
                            xp_sb[:, t0 * 128:(t0 + m) * 128]
                                .rearrange("p (j f) -> p j f", f=128),
                            p3[:, 0:m, 0:128])
                        nc.scalar.copy(
                            asad[:, t0 * 8:(t0 + m) * 8]
                                .rearrange("p (j k) -> p j k", k=8),
                            p3[:, 0:m, 128:136])

                    # w = exp(0.2*as + 0.8*relu(as+ad)), batched over the window
                    a3 = asad[:].rearrange("p (t k) -> p t k", k=8)
                    zt = wp.tile([128, NTmax * 4], F32, tag="zt")
                    nc.vector.tensor_tensor(
                        out=zt[:, 0:NT * 4].rearrange("p (t k) -> p t k", k=4),
                        in0=a3[:, 0:NT, 0:4], in1=a3[:, 0:NT, 4:8], op=OP.add)
                    zr = wp.tile([128, NTmax * 4], F32, tag="zr")
                    nc.vector.tensor_scalar(out=zr[:, 0:NT * 4], in0=zt[:, 0:NT * 4],
                                            scalar1=0.0, scalar2=4.0,
                                            op0=OP.max, op1=OP.mult)
                    t2 = wp.tile([128, NTmax * 4], F32, tag="t2")
                    nc.vector.tensor_tensor(
                        out=t2[:, 0:NT * 4].rearrange("p (t k) -> p t k", k=4),
                        in0=zr[:, 0:NT * 4].rearrange("p (t k) -> p t k", k=4),
                        in1=a3[:, 0:NT, 0:4], op=OP.add)
                    vw = wp.tile([128, NTmax * 4], F32, tag="vw")
                    nc.scalar.activation(vw[:, 0:NT * 4], t2[:, 0:NT * 4],
                                         AF.Exp, scale=0.2)

                    # XpV = [w*xp | w] per tile (bf16), on Pool + DVE
                    XpV = wp.tile([128, NTmax * 132], BF, tag="XpV")
                    X3 = XpV[:].rearrange("p (t k) -> p t k", k=132)
                    nc.gpsimd.tensor_copy(
                        X3[:, 0:NT, 128:132],
                        vw[:, 0:NT * 4].rearrange("p (t k) -> p t k", k=4))
                    for t in range(NT):
                        nc.gpsimd.tensor_tensor(
                            out=XpV[:, t * 132:t * 132 + 128]
                                .rearrange("p (h f) -> p h f", f=32),
                            in0=xp_sb[:, t * 128:(t + 1) * 128]
                                .rearrange("p (h f) -> p h f", f=32),
                            in1=vw[:, 4 * t:4 * t + 4, None].broadcast_to([128, 4, 32]),
                            op=OP.mult)

                    # aggregate: psW[c, 0:128] = sum_e w*xp ; [c,128:132] = denom
                    psW = psW_p.tile([128, 132], F32, space="PSUM", tag="psW")
                    for t in range(NT):
                        nc.tensor.matmul(psW[:], lhsT=mts[:, t * 128:(t + 1) * 128],
                                         rhs=XpV[:, t * 132:(t + 1) * 132],
                                         start=(t == 0), stop=(t == NT - 1))

                    # normalize, transpose, project, +bias, +residual, LayerNorm
                    dinv = sp.tile([128, 4], F32, tag="dinv")
                    nc.vector.reciprocal(dinv[:], psW[:, 128:132])
                    mh = sp.tile([128, 128], BF, tag="mh")
                    nc.vector.tensor_tensor(
                        out=mh[:].rearrange("p (h f) -> p h f", f=32),
                        in0=psW[:, 0:128].rearrange("p (h f) -> p h f", f=32),
                        in1=dinv[:, :, None].broadcast_to([128, 4, 32]),
                        op=OP.mult)
                    psT = psS_p.tile([128, 128], BF, space="PSUM", tag="psT")
                    nc.tensor.transpose(psT[:], mh[:], i128_sb[:])
                    mhT = sp.tile([128, 128], BF, tag="mhT")
                    nc.scalar.copy(mhT[:], psT[:])

                    psO = psO_p.tile([128, 128], F32, space="PSUM", tag="psO")
                    nc.tensor.matmul(psO[:], lhsT=mhT[:], rhs=prj_sb[:],
                                     start=True, stop=False)
                    nc.tensor.matmul(psO[:], lhsT=one1_sb[:], rhs=pb1_sb[:],
                                     start=False, stop=True)
                    tr = sp.tile([128, 128], F32, tag="tr")
                    nc.vector.tensor_add(tr[:], psO[:], xw[:])

                    # LayerNorm; 1/sqrt via bit-trick + 2 Newton steps (no ACT)
                    s1 = sp.tile([128, 1], F32, tag="s1")
                    nc.vector.tensor_reduce(s1[:], tr[:], axis=mybir.AxisListType.X,
                                            op=OP.add)
                    scr2 = sp.tile([128, 128], F32, tag="scr2")
                    nc.vector.tensor_mul(scr2[:], tr[:], tr[:])
                    q1 = sp.tile([128, 1], F32, tag="q1")
                    nc.vector.tensor_reduce(q1[:], scr2[:], axis=mybir.AxisListType.X,
                                            op=OP.add)
                    mu = sp.tile([128, 1], F32, tag="mu")
                    nc.gpsimd.tensor_scalar_mul(mu[:], s1[:], 1.0 / 128.0)
                    m2 = sp.tile([128, 1], F32, tag="m2")
                    nc.gpsimd.tensor_mul(m2[:], mu[:], mu[:])
                    qq = sp.tile([128, 1], F32, tag="qq")
                    nc.gpsimd.tensor_scalar(out=qq[:], in0=q1[:], scalar1=1.0 / 128.0,
                                            scalar2=1e-5, op0=OP.mult, op1=OP.add)
                    var = sp.tile([128, 1], F32, tag="var")
                    nc.gpsimd.tensor_sub(var[:], qq[:], m2[:])     # var + eps
                    ih = sp.tile([128, 1], I32, tag="ih")
                    nc.vector.tensor_scalar(out=ih[:], in0=var[:].bitcast(I32),
                                            scalar1=1, scalar2=None,
                                            op0=OP.arith_shift_right)
                    y0i = sp.tile([128, 1], I32, tag="y0i")
                    nc.vector.tensor_scalar(out=y0i[:], in0=ih[:], scalar1=-1,
                                            scalar2=MAGIC, op0=OP.mult, op1=OP.add)
                    vh = sp.tile([128, 1], F32, tag="vh")
                    nc.gpsimd.tensor_scalar_mul(vh[:], var[:], 0.5)
                    yy = y0i[:].bitcast(F32)
                    nwa = sp.tile([128, 1], F32, tag="nwa")
                    nwb = sp.tile([128, 1], F32, tag="nwb")
                    sv = sp.tile([128, 1], F32, tag="sv")
                    nc.gpsimd.tensor_mul(nwa[:], yy, yy)
                    nc.gpsimd.tensor_mul(nwb[:], nwa[:], vh[:])
                    nc.gpsimd.tensor_scalar(out=nwb[:], in0=nwb[:], scalar1=-1.0,
                                            scalar2=1.5, op0=OP.mult, op1=OP.add)
                    nc.gpsimd.tensor_mul(sv[:], yy, nwb[:])
                    nc.gpsimd.tensor_mul(nwa[:], sv[:], sv[:])
                    nc.gpsimd.tensor_mul(nwb[:], nwa[:], vh[:])
                    nc.gpsimd.tensor_scalar(out=nwb[:], in0=nwb[:], scalar1=-1.0,
                                            scalar2=1.5, op0=OP.mult, op1=OP.add)
                    sinv = sp.tile([128, 1], F32, tag="sinv")
                    nc.gpsimd.tensor_mul(sinv[:], sv[:], nwb[:])
                    nmu = sp.tile([128, 1], F32, tag="nmu")
                    nc.gpsimd.tensor_scalar_mul(nmu[:], mu[:], -1.0)
                    y = sp.tile([128, 128], F32, tag="y")
                    nc.vector.tensor_scalar(out=y[:], in0=tr[:], scalar1=nmu[:],
                                            scalar2=sinv[:], op0=OP.add, op1=OP.mult)
                    y2 = sp.tile([128, 128], F32, tag="y2")
                    nc.gpsimd.tensor_mul(y2[:], y[:], gb_sb[:])
                    y3 = sp.tile([128, 128], F32, tag="y3")
                    nc.gpsimd.tensor_add(y3[:], y2[:], bb_sb[:])
                    nc.sync.dma_start(out[g * 128:(g + 1) * 128, :], y3[:])
    nc.compile()
    return nc


# ---------------- host preparation ----------------
def host_prep(cfg, x, edge_index, W, a_src, a_dst, bias, proj_w, proj_b, ln_g, ln_b):
    c = cfg
    N, D = c.N, c.D
    x = np.asarray(x, np.float32)
    W = np.asarray(W, np.float32)
    a_src = np.asarray(a_src, np.float32)
    a_dst = np.asarray(a_dst, np.float32)
    bias = np.asarray(bias, np.float32)
    proj_w = np.asarray(proj_w, np.float32)
    proj_b = np.asarray(proj_b, np.float32)
    ln_g = np.asarray(ln_g, np.float32)
    ln_b = np.asarray(ln_b, np.float32)

    x16 = x.astype(ml_dtypes.bfloat16).view(np.uint16)

    wcat = W.transpose(1, 0, 2).reshape(D, D)
    ws = np.einsum("hdf,hf->dh", W, a_src)
    wd = np.einsum("hdf,hf->dh", W, a_dst)
    wcs = np.concatenate([wcat, ws], axis=1).astype(ml_dtypes.bfloat16)
    wdb = wd.astype(ml_dtypes.bfloat16)
    pb1v = (bias.reshape(D) @ proj_w + proj_b).astype(np.float32)
    i128 = np.eye(128, dtype=np.float32).astype(ml_dtypes.bfloat16)
    gbc = np.tile(ln_g, (128, 1)).astype(np.float32)
    bbc = np.tile(ln_b, (128, 1)).astype(np.float32)

    src = np.concatenate([np.asarray(edge_index[0]).astype(np.int64),
                          np.arange(N, dtype=np.int64)])
    dst = np.concatenate([np.asarray(edge_index[1]).astype(np.int64),
                          np.arange(N, dtype=np.int64)])
    order = np.argsort(dst, kind="stable")
    ds = dst[order]
    ss = src[order]

    percore = []
    allcounts = np.zeros((c.ncore, c.nwin), np.int64)
    for k in range(c.ncore):
        lo, hi = k * c.dshard, (k + 1) * c.dshard
        i0 = np.searchsorted(ds, lo)
        i1 = np.searchsorted(ds, hi)
        dsk = ds[i0:i1] - lo
        ssk = ss[i0:i1]
        win = dsk >> 7
        counts = np.bincount(win, minlength=c.nwin)
        allcounts[k] = counts
        percore.append((dsk, ssk, win, counts))

    ntg = np.maximum(1, (allcounts.max(axis=0) + 127) // 128).astype(np.int64)
    base = np.zeros(c.nwin + 1, np.int64)
    np.cumsum(ntg, out=base[1:])
    S = int(base[-1]) * 128
    ar = np.arange(128, dtype=np.float32)

    in_maps = []
    for k in range(c.ncore):
        dsk, ssk, win, counts = percore[k]
        starts = np.zeros(c.nwin + 1, np.int64)
        np.cumsum(counts, out=starts[1:])
        rank = np.arange(len(dsk)) - starts[win]
        slot = base[win] * 128 + rank

        arr = np.zeros((S, 128), np.uint16)
        arr[slot] = x16[ssk]
        xeT = np.ascontiguousarray(arr.T).view(ml_dtypes.bfloat16)

        dclf = np.full(S, -1.0, np.float32)
        dclf[slot] = (dsk & 127).astype(np.float32)
        m3 = dclf.reshape(S // 128, 128)[:, :, None] == ar[None, None, :]
        mt8 = np.ascontiguousarray(
            m3.transpose(1, 0, 2).reshape(128, S)).astype(ml_dtypes.float8_e4m3)
        mtg8 = np.ascontiguousarray(
            m3.transpose(2, 0, 1).reshape(128, S)).astype(ml_dtypes.float8_e4m3)

        lo = k * c.dshard
        hi = min(N, (k + 1) * c.dshard)
        xwin = np.zeros((c.dshard, 128), np.float32)
        xwin[:hi - lo] = x[lo:hi]
        xsT = np.ascontiguousarray(
            xwin.astype(ml_dtypes.bfloat16).view(np.uint16).T
        ).view(ml_dtypes.bfloat16)

        in_maps.append({
            "xeT": xeT,
            "mt8": mt8,
            "mtg8": mtg8,
            "x_f": xwin,
            "xsT": xsT,
            "wcs": wcs,
            "wdb": wdb,
            "prj": proj_w.astype(ml_dtypes.bfloat16),
            "pb1": pb1v.reshape(1, 128).astype(ml_dtypes.bfloat16),
            "one1": np.ones((1, 128), ml_dtypes.bfloat16),
            "i128": i128,
            "gb": gbc,
            "bb": bbc,
        })
    return in_maps, tuple(int(v) for v in ntg)


_PROG_CACHE = {}


def get_program(cfg, ntg):
    key = (cfg.N, cfg.E, cfg.dshard, tuple(ntg))
    if key not in _PROG_CACHE:
        _PROG_CACHE[key] = build_program(cfg, ntg)
    return _PROG_CACHE[key]


def kernel(x, edge_index, W, a_src, a_dst, bias, proj_w, proj_b, ln_g, ln_b):
    cfg = FULL
    in_maps, ntg = host_prep(cfg, x, edge_index, W, a_src, a_dst, bias,
                             proj_w, proj_b, ln_g, ln_b)
    nc = get_program(cfg, ntg)
    res = bass_utils.run_bass_kernel_spmd(
        nc, in_maps, core_ids=list(range(cfg.ncore)))
    out = np.zeros((cfg.N, 128), np.float32)
    for k in range(cfg.ncore):
        lo = k * cfg.dshard
        hi = min(cfg.N, (k + 1) * cfg.dshard)
        if hi > lo:
            out[lo:hi] = res.results[k]["out"][:hi - lo]
    return out


# revision 4
# speedup vs baseline: 1.0014x; 1.0014x over previous
"""Multi-head graph attention (GATConv) Trainium2 Bass kernel, v3.

v2 -> v3: one-hot Mt/MT matrices are host-built and shipped as fp8 (PE
accepts fp8 lhsT with bf16 rhs), removing on-device one-hot construction;
per-edge [alpha_src | alpha_dst] land adjacently in one PSUM tile so one
ACT copy extracts both; LayerNorm's 1/sqrt uses the bit-trick rsqrt with
two Newton iterations on Pool/DVE so the ACT engine only ever runs Exp
(single activation-table load); per-window tile counts are variable.
"""
import sys
sys.path.insert(0, "/opt/trn_rl_repo")
import numpy as np
import ml_dtypes

import concourse.bacc as bacc
import concourse.mybir as mybir
import concourse.tile as tile
from concourse import bass_utils

BF = mybir.dt.bfloat16
F8 = mybir.dt.float8e4
F32 = mybir.dt.float32
I32 = mybir.dt.int32
AF = mybir.ActivationFunctionType
OP = mybir.AluOpType

MAGIC = 0x5F3759DF


class Cfg:
    def __init__(self, N, E, ncore=8):
        self.N = N
        self.E = E
        self.D = 128
        self.H = 4
        self.HD = 32
        self.ncore = ncore
        per = (N + ncore - 1) // ncore
        self.dshard = ((per + 127) // 128) * 128
        self.nwin = self.dshard // 128


FULL = Cfg(100_000, 1_600_000)


def build_program(cfg: Cfg, ntg):
    c = cfg
    ntg = list(ntg)
    assert len(ntg) == c.nwin
    base = np.zeros(c.nwin + 1, np.int64)
    np.cumsum(ntg, out=base[1:])
    S = int(base[-1]) * 128
    NTmax = max(ntg)

    nc = bacc.Bacc("TRN2", num_devices=c.ncore, debug=False)

    xeT = nc.dram_tensor("xeT", [128, S], BF, kind="ExternalInput")
    mt8 = nc.dram_tensor("mt8", [128, S], F8, kind="ExternalInput")
    mtg8 = nc.dram_tensor("mtg8", [128, S], F8, kind="ExternalInput")
    x_f = nc.dram_tensor("x_f", [c.dshard, 128], F32, kind="ExternalInput")
    xsT = nc.dram_tensor("xsT", [128, c.dshard], BF, kind="ExternalInput")
    wcs = nc.dram_tensor("wcs", [128, 132], BF, kind="ExternalInput")
    wdb = nc.dram_tensor("wdb", [128, 4], BF, kind="ExternalInput")
    prj = nc.dram_tensor("prj", [128, 128], BF, kind="ExternalInput")
    pb1 = nc.dram_tensor("pb1", [1, 128], BF, kind="ExternalInput")
    one1 = nc.dram_tensor("one1", [1, 128], BF, kind="ExternalInput")
    i128 = nc.dram_tensor("i128", [128, 128], BF, kind="ExternalInput")
    gb = nc.dram_tensor("gb", [128, 128], F32, kind="ExternalInput")
    bb = nc.dram_tensor("bb", [128, 128], F32, kind="ExternalInput")
    out = nc.dram_tensor("out", [c.dshard, 128], F32, kind="ExternalOutput")

    with tile.TileContext(nc) as tc:
        with tc.tile_pool(name="const", bufs=1) as cp:
            wcs_sb = cp.tile([128, 132], BF)
            nc.sync.dma_start(wcs_sb[:], wcs[:])
            wdb_sb = cp.tile([128, 4], BF)
            nc.sync.dma_start(wdb_sb[:], wdb[:])
            prj_sb = cp.tile([128, 128], BF)
            nc.sync.dma_start(prj_sb[:], prj[:])
            pb1_sb = cp.tile([1, 128], BF)
            nc.sync.dma_start(pb1_sb[:], pb1[:])
            one1_sb = cp.tile([1, 128], BF)
            nc.sync.dma_start(one1_sb[:], one1[:])
            i128_sb = cp.tile([128, 128], BF)
            nc.sync.dma_start(i128_sb[:], i128[:])
            gb_sb = cp.tile([128, 128], F32)
            nc.sync.dma_start(gb_sb[:], gb[:])
            bb_sb = cp.tile([128, 128], F32)
            nc.sync.dma_start(bb_sb[:], bb[:])
            xsT_sb = cp.tile([128, c.dshard], BF)
            nc.sync.dma_start(xsT_sb[:], xsT[:])

            with (
                tc.tile_pool(name="win", bufs=2) as wp,
                tc.tile_pool(name="sc", bufs=2) as sp,
                tc.tile_pool(name="psX", bufs=3, space="PSUM") as psX_p,
                tc.tile_pool(name="psA", bufs=1, space="PSUM") as psA_p,
                tc.tile_pool(name="psW", bufs=2, space="PSUM") as psW_p,
                tc.tile_pool(name="psS", bufs=1, space="PSUM") as psS_p,
                tc.tile_pool(name="psO", bufs=1, space="PSUM") as psO_p,
            ):
                for g in range(c.nwin):
                    NT = ntg[g]
                    sl = slice(int(base[g]) * 128, int(base[g + 1]) * 128)
                    xe = wp.tile([128, NTmax * 128], BF, tag="xe")
                    nc.sync.dma_start(xe[:, 0:NT * 128], xeT[:, sl])
                    mts = wp.tile([128, NTmax * 128], F8, tag="mts")
                    nc.sync.dma_start(mts[:, 0:NT * 128], mt8[:, sl])
                    mtgs = wp.tile([128, NTmax * 128], F8, tag="mtgs")
                    nc.sync.dma_start(mtgs[:, 0:NT * 128], mtg8[:, sl])
                    xw = wp.tile([128, 128], F32, tag="xw")
                    nc.sync.dma_start(xw[:], x_f[g * 128:(g + 1) * 128, :])

                    # alpha_dst per owned dst node: [c, 4]
                    psA = psA_p.tile([128, 4], F32, space="PSUM", tag="psA")
                    nc.tensor.matmul(psA[:], lhsT=xsT_sb[:, g * 128:(g + 1) * 128],
                                     rhs=wdb_sb[:], start=True, stop=True)
                    adb = sp.tile([128, 4], BF, tag="adb")
                    nc.vector.tensor_copy(adb[:], psA[:])

                    # 3 tiles share one PSUM bank ([xp|as|ad] x3) so a single
                    # strided copy extracts xp (DVE) and [as|ad] (ACT)
                    xp_sb = wp.tile([128, NTmax * 128], BF, tag="xp_sb")
                    asad = wp.tile([128, NTmax * 8], F32, tag="asad")
                    for t0 in range(0, NT, 3):
                        m = min(3, NT - t0)
                        ps_x = psX_p.tile([128, 408], F32, space="PSUM", tag="ps_x")
                        p3 = ps_x[:].rearrange("p (j f) -> p j f", f=136)
                        for j in range(m):
                            t = t0 + j
                            nc.tensor.matmul(ps_x[:, j * 136:j * 136 + 132],
                                             lhsT=xe[:, t * 128:(t + 1) * 128],
                                             rhs=wcs_sb[:], start=True, stop=True)
                            nc.tensor.matmul(ps_x[:, j * 136 + 132:j * 136 + 136],
                                             lhsT=mtgs[:, t * 128:(t + 1) * 128],
                                             rhs=adb[:], start=True, stop=True)
                        nc.vector.tensor_copy(
                            xp_sb[:, t0 * 128:(t0 + m) * 128]
                                .rearrange("p (j f) -> p j f", f=128),
                            p3[:, 0:m, 0:128])
                        nc.scalar.copy(
                            asad[:, t0 * 8:(t0 + m) * 8]
                                .rearrange("p (j k) -> p j k", k=8),
                            p3[:, 0:m, 128:136])

                    # w = exp(0.2*as + 0.8*relu(as+ad)), batched over the window
                    a3 = asad[:].rearrange("p (t k) -> p t k", k=8)
                    zt = wp.tile([128, NTmax * 4], F32, tag="zt")
                    nc.vector.tensor_tensor(
                        out=zt[:, 0:NT * 4].rearrange("p (t k) -> p t k", k=4),
                        in0=a3[:, 0:NT, 0:4], in1=a3[:, 0:NT, 4:8], op=OP.add)
                    zr = wp.tile([128, NTmax * 4], F32, tag="zr")
                    nc.vector.tensor_scalar(out=zr[:, 0:NT * 4], in0=zt[:, 0:NT * 4],
                                            scalar1=0.0, scalar2=4.0,
                                            op0=OP.max, op1=OP.mult)
                    t2 = wp.tile([128, NTmax * 4], F32, tag="t2")
                    nc.vector.tensor_tensor(
                        out=t2[:, 0:NT * 4].rearrange("p (t k) -> p t k", k=4),
                        in0=zr[:, 0:NT * 4].rearrange("p (t k) -> p t k", k=4),
                        in1=a3[:, 0:NT, 0:4], op=OP.add)
                    vw = wp.tile([128, NTmax * 4], F32, tag="vw")
                    nc.scalar.activation(vw[:, 0:NT * 4], t2[:, 0:NT * 4],
                                         AF.Exp, scale=0.2)

                    # XpV = [w*xp | w] per tile (bf16), on Pool + DVE
                    XpV = wp.tile([128, NTmax * 132], BF, tag="XpV")
                    X3 = XpV[:].rearrange("p (t k) -> p t k", k=132)
                    nc.gpsimd.tensor_copy(
                        X3[:, 0:NT, 128:132],
                        vw[:, 0:NT * 4].rearrange("p (t k) -> p t k", k=4))
                    for t in range(NT):
                        nc.gpsimd.tensor_tensor(
                            out=XpV[:, t * 132:t * 132 + 128]
                                .rearrange("p (h f) -> p h f", f=32),
                            in0=xp_sb[:, t * 128:(t + 1) * 128]
                                .rearrange("p (h f) -> p h f", f=32),
                            in1=vw[:, 4 * t:4 * t + 4, None].broadcast_to([128, 4, 32]),
                            op=OP.mult)

                    # aggregate: psW[c, 0:128] = sum_e w*xp ; [c,128:132] = denom
                    psW = psW_p.tile([128, 132], F32, space="PSUM", tag="psW")
                    for t in range(NT):
                        nc.tensor.matmul(psW[:], lhsT=mts[:, t * 128:(t + 1) * 128],
                                         rhs=XpV[:, t * 132:(t + 1) * 132],
                                         start=(t == 0), stop=(t == NT - 1))

                    # normalize, transpose, project, +bias, +residual, LayerNorm
                    dinv = sp.tile([128, 4], F32, tag="dinv")
                    nc.vector.reciprocal(dinv[:], psW[:, 128:132])
                    mh = sp.tile([128, 128], BF, tag="mh")
                    nc.vector.tensor_tensor(
                        out=mh[:].rearrange("p (h f) -> p h f", f=32),
                        in0=psW[:, 0:128].rearrange("p (h f) -> p h f", f=32),
                        in1=dinv[:, :, None].broadcast_to([128, 4, 32]),
                        op=OP.mult)
                    psT = psS_p.tile([128, 128], BF, space="PSUM", tag="psT")
                    nc.tensor.transpose(psT[:], mh[:], i128_sb[:])
                    mhT = sp.tile([128, 128], BF, tag="mhT")
                    nc.scalar.copy(mhT[:], psT[:])

                    psO = psO_p.tile([128, 128], F32, space="PSUM", tag="psO")
                    nc.tensor.matmul(psO[:], lhsT=mhT[:], rhs=prj_sb[:],
                                     start=True, stop=False)
                    nc.tensor.matmul(psO[:], lhsT=one1_sb[:], rhs=pb1_sb[:],
                                     start=False, stop=True)
                    tr = sp.tile([128, 128], F32, tag="tr")
                    nc.vector.tensor_add(tr[:], psO[:], xw[:])

                    # LayerNorm; 1/sqrt via bit-trick + 2 Newton steps (no ACT)
                    s1 = sp.tile([128, 1], F32, tag="s1")
                    nc.vector.tensor_reduce(s1[:], tr[:], axis=mybir.AxisListType.X,
                                            op=OP.add)
                    scr2 = sp.tile([128, 128], F32, tag="scr2")
                    nc.vector.tensor_mul(scr2[:], tr[:], tr[:])
                    q1 = sp.tile([128, 1], F32, tag="q1")
                    nc.vector.tensor_reduce(q1[:], scr2[:], axis=mybir.AxisListType.X,
                                            op=OP.add)
                    mu = sp.tile([128, 1], F32, tag="mu")
                    nc.gpsimd.tensor_scalar_mul(mu[:], s1[:], 1.0 / 128.0)
                    m2 = sp.tile([128, 1], F32, tag="m2")
                    nc.gpsimd.tensor_mul(m2[:], mu[:], mu[:])
                    qq = sp.tile([128, 1], F32, tag="qq")
                    nc.gpsimd.tensor_scalar(out=qq[:], in0=q1[:], scalar1=1.0 / 128.0,
                                            scalar2=1e-5, op0=OP.mult, op1=OP.add)
                    var = sp.tile([128, 1], F32, tag="var")
                    nc.gpsimd.tensor_sub(var[:], qq[:], m2[:])     # var + eps
                    ih = sp.tile([128, 1], I32, tag="ih")
                    nc.vector.tensor_scalar(out=ih[:], in0=var[:].bitcast(I32),
                                            scalar1=1, scalar2=None,
                                            op0=OP.arith_shift_right)
                    y0i = sp.tile([128, 1], I32, tag="y0i")
                    nc.vector.tensor_scalar(out=y0i[:], in0=ih[:], scalar1=-1,
                                            scalar2=MAGIC, op0=OP.mult, op1=OP.add)
                    vh = sp.tile([128, 1], F32, tag="vh")
                    nc.gpsimd.tensor_scalar_mul(vh[:], var[:], 0.5)
                    yy = y0i[:].bitcast(F32)
                    nwa = sp.tile([128, 1], F32, tag="nwa")
                    nwb = sp.tile([128, 1], F32, tag="nwb")
                    sv = sp.tile([128, 1], F32, tag="sv")
                    nc.gpsimd.tensor_mul(nwa[:], yy, yy)
                    nc.gpsimd.tensor_mul(nwb[:], nwa[:], vh[:])
                    nc.gpsimd.tensor_scalar(out=nwb[:], in0=nwb[:], scalar1=-1.0,
                                            scalar2=1.5, op0=OP.mult, op1=OP.add)
                    nc.gpsimd.tensor_mul(sv[:], yy, nwb[:])
                    nc.gpsimd.tensor_mul(nwa[:], sv[:], sv[:])
                    nc.gpsimd.tensor_mul(nwb[:], nwa[:], vh[:])
                    nc.gpsimd.tensor_scalar(out=nwb[:], in0=nwb[:], scalar1=-1.0,
                                            scalar2=1.5, op0=OP.mult, op1=OP.add)
                    sinv = sp.tile([128, 1], F32, tag="sinv")
                    nc.gpsimd.tensor_mul(sinv[:], sv[:], nwb[:])
                    nmu = sp.tile([128, 1], F32, tag="nmu")
                    nc.gpsimd.tensor_scalar_mul(nmu[:], mu[:], -1.0)
                    y = sp.tile([128, 128], F32, tag="y")
                    nc.vector.tensor_scalar(out=y[:], in0=tr[:], scalar1=nmu[:],
                                            scalar2=sinv[:], op0=OP.add, op1=OP.mult)
                    y2 = sp.tile([128, 128], F32, tag="y2")
                    nc.gpsimd.tensor_mul(y2[:], y[:], gb_sb[:])
                    y3 = sp.tile([128, 128], F32, tag="y3")
                    nc.gpsimd.tensor_add(y3[:], y2[:], bb_sb[:])
                    nc.sync.dma_start(out[g * 128:(g + 1) * 128, :], y3[:])
    nc.compile()
    return nc


# ---------------- host preparation ----------------
def host_prep(cfg, x, edge_index, W, a_src, a_dst, bias, proj_w, proj_b, ln_g, ln_b):
    c = cfg
    N, D = c.N, c.D
    x = np.asarray(x, np.float32)
    W = np.asarray(W, np.float32)
    a_src = np.asarray(a_src, np.float32)
    a_dst = np.asarray(a_dst, np.float32)
    bias = np.asarray(bias, np.float32)
    proj_w = np.asarray(proj_w, np.float32)
    proj_b = np.asarray(proj_b, np.float32)
    ln_g = np.asarray(ln_g, np.float32)
    ln_b = np.asarray(ln_b, np.float32)

    x16 = x.astype(ml_dtypes.bfloat16).view(np.uint16)

    wcat = W.transpose(1, 0, 2).reshape(D, D)
    ws = np.einsum("hdf,hf->dh", W, a_src)
    wd = np.einsum("hdf,hf->dh", W, a_dst)
    wcs = np.concatenate([wcat, ws], axis=1).astype(ml_dtypes.bfloat16)
    wdb = wd.astype(ml_dtypes.bfloat16)
    pb1v = (bias.reshape(D) @ proj_w + proj_b).astype(np.float32)
    i128 = np.eye(128, dtype=np.float32).astype(ml_dtypes.bfloat16)
    gbc = np.tile(ln_g, (128, 1)).astype(np.float32)
    bbc = np.tile(ln_b, (128, 1)).astype(np.float32)

    src = np.concatenate([np.asarray(edge_index[0]).astype(np.int64),
                          np.arange(N, dtype=np.int64)])
    dst = np.concatenate([np.asarray(edge_index[1]).astype(np.int64),
                          np.arange(N, dtype=np.int64)])
    order = np.argsort(dst, kind="stable")
    ds = dst[order]
    ss = src[order]

    import heapq
    percore = []
    allcounts = np.zeros((c.ncore, c.nwin), np.int64)
    for k in range(c.ncore):
        lo, hi = k * c.dshard, (k + 1) * c.dshard
        i0 = np.searchsorted(ds, lo)
        i1 = np.searchsorted(ds, hi)
        dsk = ds[i0:i1] - lo
        ssk = ss[i0:i1]
        # balance edge counts across windows: greedy LPT with exactly 128
        # dsts per window (output rows are un-permuted on the host)
        deg = np.bincount(dsk, minlength=c.dshard)
        order_d = np.argsort(-deg, kind="stable")
        heap = [(0, 0, w) for w in range(c.nwin)]
        win_of = np.empty(c.dshard, np.int32)
        col_of = np.empty(c.dshard, np.int32)
        for d in order_d:
            while True:
                s, cnt, w = heapq.heappop(heap)
                if cnt < 128:
                    break
            win_of[d] = w
            col_of[d] = cnt
            heapq.heappush(heap, (s + int(deg[d]), cnt + 1, w))
        # swap-repair: one spill window absorbs the overflow so the other
        # windows stay at ceil(mean/128) tiles
        total = int(deg.sum())
        CAP = (total // c.nwin // 128) * 128       # floor to tile multiple
        if total - (c.nwin - 1) * CAP > 40 * 128:  # spill would blow up
            CAP += 128
        members = [list(np.where(win_of == w)[0]) for w in range(c.nwin)]
        sums = np.zeros(c.nwin, np.int64)
        np.add.at(sums, win_of, deg)
        spill = int(np.argmax(sums))
        for _ in range(5000):
            tmp = sums.copy()
            tmp[spill] = -1
            hi = int(np.argmax(tmp))
            if sums[hi] <= CAP:
                break
            need = int(sums[hi] - CAP)
            mh = np.array(members[hi])
            ms = np.array(members[spill])
            diff = deg[mh][:, None].astype(np.int64) - deg[ms][None, :]
            ok = diff >= need
            if not ok.any():
                break
            masked = np.where(ok, diff, 1 << 40)
            i, j = np.unravel_index(int(np.argmin(masked)), diff.shape)
            a, b = int(mh[i]), int(ms[j])
            members[hi][i] = b
            members[spill][j] = a
            delta = int(deg[a] - deg[b])
            sums[hi] -= delta
            sums[spill] += delta
        for w in range(c.nwin):
            for col, d in enumerate(members[w]):
                win_of[d] = w
                col_of[d] = col
        # relabel windows heaviest-first so overflow windows align across cores
        wsum = np.zeros(c.nwin, np.int64)
        np.add.at(wsum, win_of, deg)
        relab = np.empty(c.nwin, np.int32)
        relab[np.argsort(-wsum, kind="stable")] = np.arange(c.nwin)
        win_of = relab[win_of]
        perm = np.empty(c.dshard, np.int64)          # row slot -> local dst id
        perm[win_of.astype(np.int64) * 128 + col_of] = np.arange(c.dshard)
        win = win_of[dsk]
        counts = np.bincount(win, minlength=c.nwin)
        allcounts[k] = counts
        percore.append((dsk, ssk, win, counts, win_of, col_of, perm))

    ntg = np.maximum(1, (allcounts.max(axis=0) + 127) // 128).astype(np.int64)
    base = np.zeros(c.nwin + 1, np.int64)
    np.cumsum(ntg, out=base[1:])
    S = int(base[-1]) * 128
    ar = np.arange(128, dtype=np.float32)

    in_maps = []
    perms = []
    for k in range(c.ncore):
        dsk, ssk, win, counts, win_of, col_of, perm = percore[k]
        perms.append(perm)
        starts = np.zeros(c.nwin + 1, np.int64)
        np.cumsum(counts, out=starts[1:])
        order2 = np.argsort(win, kind="stable")
        dsk = dsk[order2]
        ssk = ssk[order2]
        win = win[order2]
        rank = np.arange(len(dsk)) - starts[win]
        slot = base[win] * 128 + rank

        arr = np.zeros((S, 128), np.uint16)
        arr[slot] = x16[ssk]
        xeT = np.ascontiguousarray(arr.T).view(ml_dtypes.bfloat16)

        dclf = np.full(S, -1.0, np.float32)
        dclf[slot] = col_of[dsk].astype(np.float32)
        m3 = dclf.reshape(S // 128, 128)[:, :, None] == ar[None, None, :]
        mt8 = np.ascontiguousarray(
            m3.transpose(1, 0, 2).reshape(128, S)).astype(ml_dtypes.float8_e4m3)
        mtg8 = np.ascontiguousarray(
            m3.transpose(2, 0, 1).reshape(128, S)).astype(ml_dtypes.float8_e4m3)

        lo = k * c.dshard
        hi = min(N, (k + 1) * c.dshard)
        xfull = np.zeros((c.dshard, 128), np.float32)
        xfull[:hi - lo] = x[lo:hi]
        xwin = xfull[perm]                      # row-slot order
        xsT = np.ascontiguousarray(
            xwin.astype(ml_dtypes.bfloat16).view(np.uint16).T
        ).view(ml_dtypes.bfloat16)

        in_maps.append({
            "xeT": xeT,
            "mt8": mt8,
            "mtg8": mtg8,
            "x_f": xwin,
            "xsT": xsT,
            "wcs": wcs,
            "wdb": wdb,
            "prj": proj_w.astype(ml_dtypes.bfloat16),
            "pb1": pb1v.reshape(1, 128).astype(ml_dtypes.bfloat16),
            "one1": np.ones((1, 128), ml_dtypes.bfloat16),
            "i128": i128,
            "gb": gbc,
            "bb": bbc,
        })
    return in_maps, tuple(int(v) for v in ntg), perms


_PROG_CACHE = {}


def get_program(cfg, ntg):
    key = (cfg.N, cfg.E, cfg.dshard, tuple(ntg))
    if key not in _PROG_CACHE:
        _PROG_CACHE[key] = build_program(cfg, ntg)
    return _PROG_CACHE[key]


def kernel(x, edge_index, W, a_src, a_dst, bias, proj_w, proj_b, ln_g, ln_b):
    cfg = FULL
    in_maps, ntg, perms = host_prep(cfg, x, edge_index, W, a_src, a_dst,
                                    bias, proj_w, proj_b, ln_g, ln_b)
    nc = get_program(cfg, ntg)
    res = bass_utils.run_bass_kernel_spmd(
        nc, in_maps, core_ids=list(range(cfg.ncore)))
    out = np.zeros((cfg.N, 128), np.float32)
    for k in range(cfg.ncore):
        lo = k * cfg.dshard
        gid = lo + perms[k]
        valid = gid < cfg.N
        out[gid[valid]] = res.results[k]["out"][valid]
    return out
